# revision 10
# baseline (speedup 1.0000x reference)
"""Trainium2 Bass kernel for batched single-head attention with projections.

Reference computation (per batch b):
    Q = q @ Wq + bq ; K = k @ Wk + bk ; V = v @ Wv + bv        (512 -> 64)
    out = softmax(Q K^T / 8) V                                  (S = 4096)

Sharding: 8 cores = 4 batches x 2 kv-sequence halves. Each core gets the
full q for its batch plus its half of k,v (q/k in fp8-e3m4, v in bf16),
all host-swizzled into the exact [128, chunk, cols] SBUF layout so every
DMA is a flat contiguous per-partition transfer.

Device-side layout trick: everything is computed in "transposed space".
  Q.T [128, 4096] = [Wq|Wq].T @ qT (+bq)   rows 64..127 duplicate 0..63
  K.T [128, 2048] = [Wk|Wk].T @ kT         (bk dropped: softmax-invariant)
  V'  [2048, 65]  = (vT.T @ Wv_aug) + bias ; col 64 == 1.0 (denominator)
  scores.T tile   = K.T-chunk.T @ Q.T-block     -> PSUM [128, 1024]
  P.T             = exp(scores.T / 8)           -> SBUF bf16 (ScalarE)
  out.T [65, 512] = sum_t V'-tile.T @ P.T-tile  -> PSUM accumulate
Rows 0..63 of out.T are the unnormalized numerator, row 64 the softmax
denominator; the host divides and transposes while unsharding.

The scores matmul has contraction dim 64, so pairs of kv-tiles are packed
into the two 64-row halves of the PE array (tile_position row tiling) and
run concurrently. The projections use doubled [W|W] stationaries so one
matmul writes both partition halves. The 64 (scores -> exp -> AV) pair
steps are emitted software-pipelined with a 1-step skew so the PE is
never queued behind an exp it could be running ahead of.
"""

import numpy as np
import ml_dtypes

import concourse.bass as bass
import concourse.tile as tile
from concourse import mybir
from concourse.bass_utils import run_bass_kernel_spmd
from concourse.tile import add_dep_helper

BF16 = mybir.dt.bfloat16
F32 = mybir.dt.float32
FP8 = mybir.dt.float8e3   # e3m4: 4 mantissa bits, max 15.5 — fits randn

B, S, D, E = 4, 4096, 512, 64
H = S                 # q rows per core (full sequence)
KS = S // 2           # kv rows per core (half sequence)
E1 = E + 1            # V' width (ones column appended)
E2 = 2 * E            # doubled projection width ([W|W] stationary)
NCH = D // 128        # contraction chunks (4)
NKV = KS // 128       # kv tiles (16)
NPAIR = NKV // 2      # packed kv tile pairs (8)
QBLK = 512            # sq columns per block
NBLK = H // QBLK      # 8
NSTEP = NBLK * NPAIR  # 64 pipelined pair steps
N_CORES = 8

# (name, dtype, n_cols) for the chunked input loads; q/k ride the sync
# ring in this order, v rides scalar (delayed behind ka, see _body)
Q_SPLITS = (("qa", 512), ("qb", 1536), ("qc", 2048))
K_SPLITS = (("ka", 512), ("kb", 512), ("kc", 1024))
V_SPLITS = (("va", 1024), ("vb", 1024))


def _build_bass(split_waits: bool = True) -> bass.Bass:
    nc = bass.Bass()
    qk_parms = {}
    for name, w in Q_SPLITS + K_SPLITS:
        qk_parms[name] = nc.declare_dram_parameter(name, [128, NCH, w], FP8,
                                                   isOutput=False)
    for name, w in V_SPLITS:
        qk_parms[name] = nc.declare_dram_parameter(name, [128, NCH, w], BF16,
                                                   isOutput=False)
    wq = nc.declare_dram_parameter("wq", [128, NCH * E2], BF16, isOutput=False)
    wk = nc.declare_dram_parameter("wk", [128, NCH * E2], BF16, isOutput=False)
    wv = nc.declare_dram_parameter("wv", [128, NCH * E1], BF16, isOutput=False)
    bq = nc.declare_dram_parameter("bqb", [128, 512], BF16, isOutput=False)
    bvb = nc.declare_dram_parameter("bvb", [128, E1], F32, isOutput=False)
    out = nc.declare_dram_parameter("out", [E1, H], F32, isOutput=True)

    with tile.TileContext(nc) as tc:
        _body(nc, tc, qk_parms, wq, wk, wv, bq, bvb, out)
    if split_waits:
        _split_multi_waits(nc)
    return nc


_NO_SPLIT_OPCODES = {"Drain", "EventSemaphore", "NoOp", "Call", "ISA",
                     "UnconditionalBranch"}


def _split_multi_waits(nc):
    """walrus (this toolchain) encodes at most ONE sem wait per TPB
    instruction (single NEURON_ISA_TPB_EVENTS slot) and refuses to compile
    instructions carrying more. Tile emits multi-wait sync_info freely, so
    split: keep the first wait on the instruction, hoist the rest onto
    standalone EventSemaphore waits just before it on the same engine."""
    n = 0
    for blk in nc.m.functions[0].blocks:
        new_insts = []
        for inst in blk.instructions:
            si = inst.sync_info
            if (si is not None and si.on_wait and len(si.on_wait) > 1
                    and inst.concise_opcode not in _NO_SPLIT_OPCODES):
                waits = list(si.on_wait)
                for w in waits[:-1]:
                    n += 1
                    es = mybir.InstEventSemaphore(
                        name=f"WSPLIT-{n}", ins=[], outs=[])
                    es.engine = inst.engine
                    es.sync_info = mybir.SyncInfo(on_wait=[w], on_update=[])
                    new_insts.append(es)
                inst.sync_info = mybir.SyncInfo(
                    on_wait=[waits[-1]], on_update=list(si.on_update))
            new_insts.append(inst)
        blk.instructions = new_insts


def _body(nc, tc, parms, wq, wk, wv, bq, bvb, out):
    with (
        tc.tile_pool(name="consts", bufs=1) as cst,
        tc.tile_pool(name="raw", bufs=1) as raw,
        tc.tile_pool(name="proj", bufs=1) as proj,
        tc.tile_pool(name="pt", bufs=8) as ptp,
        tc.tile_pool(name="ob", bufs=2) as obp,
        tc.tile_pool(name="ps", bufs=2, space="PSUM") as ps,
        tc.tile_pool(name="psc", bufs=2, space="PSUM") as psc,
        tc.tile_pool(name="pso", bufs=2, space="PSUM") as pso,
    ):
        # consts ride the gpsimd SWDGE ring so they never add fixed
        # per-DMA latency ahead of the big input loads on the HWDGE rings
        wk_sb = cst.tile([128, NCH * E2], BF16, tag="wk")
        nc.gpsimd.dma_start(out=wk_sb, in_=wk[:, :])
        wq_sb = cst.tile([128, NCH * E2], BF16, tag="wq")
        nc.gpsimd.dma_start(out=wq_sb, in_=wq[:, :])
        bq_sb = cst.tile([128, 512], BF16, tag="bq")
        nc.gpsimd.dma_start(out=bq_sb, in_=bq[:, :])
        wv_sb = cst.tile([128, NCH * E1], BF16, tag="wv")
        nc.gpsimd.dma_start(out=wv_sb, in_=wv[:, :])
        bvb_sb = cst.tile([128, E1], F32, tag="bvb")
        nc.gpsimd.dma_start(out=bvb_sb, in_=bvb[:, :])
        # preload the exp table set off the critical path (first real exp
        # otherwise eats the ~2.7us ACT_TABLE_LOAD mid-pipeline)
        scr = cst.tile([1, 8], F32, tag="scr")
        nc.scalar.activation(scr[:, :], bvb_sb[0:1, 0:8],
                             mybir.ActivationFunctionType.Exp)

        # raw inputs, host-swizzled to the SBUF layout: each DMA is a
        # flat [128, chunk*cols] contiguous-per-partition transfer.
        # sync carries qT+kT (critical chain to the first scores matmul);
        # scalar carries vT, gated behind ka so v doesn't steal SDMA
        # bandwidth from the first projections.
        tiles = {}
        insts = {}

        def load(eng, name, w, dt):
            t = raw.tile([128, NCH, w], dt, tag=name)
            insts[name] = eng.dma_start(out=t, in_=parms[name][:, :, :])
            tiles[name] = t

        load(nc.sync, "qa", 512, FP8)
        load(nc.sync, "ka", 512, FP8)
        load(nc.sync, "kb", 512, FP8)
        load(nc.sync, "kc", 1024, FP8)
        load(nc.sync, "qb", 1536, FP8)
        load(nc.sync, "qc", 2048, FP8)
        load(nc.scalar, "va", 1024, BF16)
        load(nc.scalar, "vb", 1024, BF16)
        add_dep_helper(insts["va"].ins, insts["ka"].ins, True,
                       "delay v behind ka")

        def qt_slice(c, blk):
            if blk == 0:
                return tiles["qa"][:, c, :]
            if blk < 4:
                return tiles["qb"][:, c, (blk - 1) * 512:blk * 512]
            return tiles["qc"][:, c, (blk - 4) * 512:(blk - 3) * 512]

        def kt_slice(c, blk):       # 512-col K projection block
            if blk == 0:
                return tiles["ka"][:, c, :]
            if blk == 1:
                return tiles["kb"][:, c, :]
            return tiles["kc"][:, c, (blk - 2) * 512:(blk - 1) * 512]

        def vt_slice(c, t):         # 128-col V tile
            return tiles["va" if t < 8 else "vb"][:, c,
                                                  (t % 8) * 128:(t % 8 + 1) * 128]

        # projected tensors; Q.T/K.T have rows 0..63 duplicated into
        # 64..127 (written in one pass via the doubled stationaries) so
        # the scores matmuls can row-pack both PE array halves
        QT2 = proj.tile([128, H], BF16, tag="QT2")
        KT2 = proj.tile([128, KS], BF16, tag="KT2")
        Vp = proj.tile([128, NKV, E1], BF16, tag="Vp")

        def q_proj(blk):
            acc = ps.tile([128, 512], F32, tag="ps_main")
            sl = slice(blk * 512, (blk + 1) * 512)
            for c in range(NCH):
                nc.tensor.matmul(
                    acc[:, :], wq_sb[:, c * E2:(c + 1) * E2], qt_slice(c, blk),
                    start=(c == 0), stop=(c == NCH - 1),
                )
            nc.vector.tensor_add(QT2[:, sl], acc[:, :], bq_sb[:, :])

        def k_proj(blk):
            acc = ps.tile([128, 512], F32, tag="ps_main")
            sl = slice(blk * 512, (blk + 1) * 512)
            for c in range(NCH):
                nc.tensor.matmul(
                    acc[:, :], wk_sb[:, c * E2:(c + 1) * E2], kt_slice(c, blk),
                    start=(c == 0), stop=(c == NCH - 1),
                )
            nc.vector.tensor_copy(KT2[:, sl], acc[:, :])

        def v_proj(t):
            acc = ps.tile([128, E1], F32, tag="ps_main")
            for c in range(NCH):
                nc.tensor.matmul(
                    acc[:, :], vt_slice(c, t),
                    wv_sb[:, c * E1:(c + 1) * E1],
                    start=(c == 0), stop=(c == NCH - 1),
                )
            nc.vector.tensor_add(Vp[:, t, :], acc[:, :], bvb_sb[:, :])

        # ---- software-pipelined attention: 64 pair steps, skew 1 ----
        sc_t = [None] * NSTEP
        pt_t = [None] * NSTEP
        acc = {}

        def scores(i):
            blk, p = divmod(i, NPAIR)
            sq = slice(blk * QBLK, (blk + 1) * QBLK)
            sc = psc.tile([128, 2 * QBLK], F32, tag="ps_sc")
            nc.tensor.matmul(
                sc[:, 0:QBLK],
                KT2[0:E, (2 * p) * 128:(2 * p + 1) * 128],
                QT2[0:E, sq],
                start=True, stop=True, tile_position=(0, 0),
            )
            nc.tensor.matmul(
                sc[:, QBLK:2 * QBLK],
                KT2[E:2 * E, (2 * p + 1) * 128:(2 * p + 2) * 128],
                QT2[E:2 * E, sq],
                start=True, stop=True, tile_position=(64, 0),
            )
            sc_t[i] = sc

        def expq(i):
            pt = ptp.tile([128, 2 * QBLK], BF16, tag="pt")
            nc.scalar.activation(
                pt[:, :], sc_t[i][:, :], mybir.ActivationFunctionType.Exp,
                scale=0.125,
            )
            pt_t[i] = pt

        def av(i):
            blk, p = divmod(i, NPAIR)
            if p == 0:
                acc[blk] = pso.tile([E1, QBLK], F32, tag="ps_out",
                                    name=f"acc{blk}")
            a = acc[blk]
            nc.tensor.matmul(
                a[:, :], Vp[:, 2 * p, :], pt_t[i][:, 0:QBLK],
                start=(p == 0), stop=False,
            )
            nc.tensor.matmul(
                a[:, :], Vp[:, 2 * p + 1, :], pt_t[i][:, QBLK:2 * QBLK],
                start=False, stop=(p == NPAIR - 1),
            )
            if p == NPAIR - 1:
                sq = slice(blk * QBLK, (blk + 1) * QBLK)
                ob = obp.tile([E1, QBLK], F32, tag="ob")
                nc.vector.tensor_copy(ob[:, :], a[:, :])
                nc.scalar.dma_start(out=out[:, sq], in_=ob[:, :])

        # filler (projection) work attached ahead of specific steps so it
        # lands in the PE's exp-wait slack and tracks DMA arrival order
        fillers = {
            0: [lambda: v_proj(2), lambda: v_proj(3)],
            1: [lambda: k_proj(1), lambda: v_proj(4), lambda: v_proj(5)],
            2: [lambda: v_proj(6), lambda: v_proj(7)],
            3: [lambda: k_proj(2), lambda: v_proj(8), lambda: v_proj(9)],
            4: [lambda: q_proj(1), lambda: v_proj(10), lambda: v_proj(11)],
            5: [lambda: k_proj(3), lambda: v_proj(12), lambda: v_proj(13)],
            6: [lambda: v_proj(14), lambda: v_proj(15)],
        }
        for b in range(2, NBLK):
            fillers.setdefault(8 * (b - 1) + 4, []).append(
                lambda b=b: q_proj(b))

        k_proj(0)
        q_proj(0)
        v_proj(0)
        v_proj(1)
        scores(0)
        expq(0)
        for f in fillers.get(0, []):
            f()
        for i in range(1, NSTEP):
            scores(i)
            expq(i)
            for f in fillers.get(i, []):
                f()
            av(i - 1)
        av(NSTEP - 1)


_CACHED_NC = None


def _get_nc():
    global _CACHED_NC
    if _CACHED_NC is None:
        _CACHED_NC = _build_bass()
    return _CACHED_NC


def _swizzle_w(w: np.ndarray, double: bool = False) -> np.ndarray:
    """[512, width] -> [128, NCH*width] with chunk-major free dim.
    double=True emits [W|W] chunks ([128, NCH*2*width]) so one matmul
    writes the projection into both partition halves."""
    width = w.shape[1]
    c = w.reshape(NCH, 128, width)
    if double:
        c = np.concatenate([c, c], axis=2)
        width *= 2
    return np.ascontiguousarray(
        c.transpose(1, 0, 2).reshape(128, NCH * width)
    ).astype(ml_dtypes.bfloat16)


def _chunk3d(xT: np.ndarray, splits, dt) -> dict:
    """[512, N] (transposed input) -> per-split [128, NCH, w] arrays in
    the exact SBUF tile layout (partition p, chunk c) = row c*128+p."""
    x = np.asarray(xT, np.float32).reshape(NCH, 128, xT.shape[1])
    outmaps = {}
    c0 = 0
    for name, w in splits:
        outmaps[name] = np.ascontiguousarray(
            x[:, :, c0:c0 + w].transpose(1, 0, 2)).astype(dt)
        c0 += w
    return outmaps


def _make_in_maps(q, k, v, Wq, bq, Wk, bk, Wv, bv):
    del bk  # constant along the kv axis -> softmax-invariant, dropped
    bf = ml_dtypes.bfloat16
    f8 = ml_dtypes.float8_e3m4
    wq_s = _swizzle_w(np.asarray(Wq, np.float32), double=True)
    wk_s = _swizzle_w(np.asarray(Wk, np.float32), double=True)
    wv_aug = np.concatenate(
        [np.asarray(Wv, np.float32), np.zeros((D, 1), np.float32)], axis=1
    )
    wv_s = _swizzle_w(wv_aug)
    bq_col = np.asarray(bq, np.float32).reshape(E, 1)
    bq_a = np.ascontiguousarray(np.broadcast_to(
        np.concatenate([bq_col, bq_col], axis=0), (2 * E, 512))).astype(bf)
    bvb_row = np.concatenate([np.asarray(bv, np.float32), [1.0]]).astype(np.float32)
    bvb_a = np.ascontiguousarray(np.broadcast_to(bvb_row, (128, E1)))

    in_maps = []
    for core in range(N_CORES):
        b, h = core // 2, core % 2
        m = {"wq": wq_s, "wk": wk_s, "wv": wv_s, "bqb": bq_a, "bvb": bvb_a}
        m.update(_chunk3d(np.asarray(q[b], np.float32).T, Q_SPLITS, f8))
        m.update(_chunk3d(
            np.asarray(k[b, h * KS:(h + 1) * KS, :], np.float32).T,
            K_SPLITS, f8))
        m.update(_chunk3d(
            np.asarray(v[b, h * KS:(h + 1) * KS, :], np.float32).T,
            V_SPLITS, bf))
        in_maps.append(m)
    return in_maps


def _unshard(results) -> np.ndarray:
    final = np.empty((B, S, E), np.float32)
    for b in range(B):
        o = (np.asarray(results[2 * b]["out"], np.float32)
             + np.asarray(results[2 * b + 1]["out"], np.float32))  # [65, S]
        final[b] = (o[:E] / o[E:E + 1]).T
    return final


def kernel(q, k, v, Wq, bq, Wk, bk, Wv, bv, _trace=False):
    nc = _get_nc()
    in_maps = _make_in_maps(q, k, v, Wq, bq, Wk, bk, Wv, bv)
    res = run_bass_kernel_spmd(nc, in_maps, core_ids=list(range(N_CORES)),
                               trace=_trace)
    outp = _unshard(res.results)
    if _trace:
        kernel.last_result = res
    return outp


# revision 15
# speedup vs baseline: 1.0512x; 1.0512x over previous
"""Trainium2 Bass kernel for batched single-head attention with projections.

Reference computation (per batch b):
    Q = q @ Wq + bq ; K = k @ Wk + bk ; V = v @ Wv + bv        (512 -> 64)
    out = softmax(Q K^T / 8) V                                  (S = 4096)

Sharding: 8 cores = 4 batches x 2 kv-sequence halves. Each core gets the
full q for its batch plus its half of k,v (q/k in fp8-e3m4, v in bf16),
all host-swizzled into the exact [128, chunk, cols] SBUF layout so every
DMA is a flat contiguous per-partition transfer.

Device-side layout trick: everything is computed in "transposed space".
  Q.T [128, 4096] = [Wq|Wq].T @ qT (+bq)   rows 64..127 duplicate 0..63
  K.T [128, 2048] = [Wk|Wk].T @ kT         (bk dropped: softmax-invariant)
  V'  [2048, 65]  = (vT.T @ Wv_aug) + bias ; col 64 == 1.0 (denominator)
  scores.T tile   = K.T-chunk.T @ Q.T-block     -> PSUM [128, 1024]
  P.T             = exp(scores.T / 8)           -> SBUF bf16 (ScalarE)
  out.T [65, 512] = sum_t V'-tile.T @ P.T-tile  -> PSUM accumulate
Rows 0..63 of out.T are the unnormalized numerator, row 64 the softmax
denominator; the host divides and transposes while unsharding.

The scores matmul has contraction dim 64, so pairs of kv-tiles are packed
into the two 64-row halves of the PE array (tile_position row tiling) and
run concurrently. The projections use doubled [W|W] stationaries so one
matmul writes both partition halves. The 64 (scores -> exp -> AV) pair
steps are emitted software-pipelined with a 1-step skew so the PE is
never queued behind an exp it could be running ahead of.
"""

import numpy as np
import ml_dtypes

import concourse.bass as bass
import concourse.tile as tile
from concourse import mybir
from concourse.bass_utils import run_bass_kernel_spmd
from concourse.tile import add_dep_helper

BF16 = mybir.dt.bfloat16
F32 = mybir.dt.float32
FP8 = mybir.dt.float8e3   # e3m4: 4 mantissa bits, max 15.5 — fits randn

B, S, D, E = 4, 4096, 512, 64
H = S                 # q rows per core (full sequence)
KS = S // 2           # kv rows per core (half sequence)
E1 = E + 1            # V' width (ones column appended)
E2 = 2 * E            # doubled projection width ([W|W] stationary)
NCH = D // 128        # contraction chunks (4)
NKV = KS // 128       # kv tiles (16)
NPAIR = NKV // 2      # packed kv tile pairs (8)
QBLK = 512            # sq columns per block
NBLK = H // QBLK      # 8
NSTEP = NBLK * NPAIR  # 64 pipelined pair steps
N_CORES = 8

# (name, dtype, n_cols) for the chunked input loads; q/k ride the sync
# ring in this order, v rides scalar
Q_SPLITS = (("qa", 512), ("qb", 1536), ("qc", 2048))
K_SPLITS = (("ka", 512), ("kb", 512), ("kc", 1024))
V_SPLITS = (("va", 512), ("vb", 1536))


def _build_bass(split_waits: bool = True) -> bass.Bass:
    nc = bass.Bass()
    qk_parms = {}
    for name, w in Q_SPLITS + K_SPLITS:
        qk_parms[name] = nc.declare_dram_parameter(name, [128, NCH, w], FP8,
                                                   isOutput=False)
    for name, w in V_SPLITS:
        qk_parms[name] = nc.declare_dram_parameter(name, [128, NCH, w], BF16,
                                                   isOutput=False)
    wq = nc.declare_dram_parameter("wq", [128, NCH * E2], BF16, isOutput=False)
    wk = nc.declare_dram_parameter("wk", [128, NCH * E2], BF16, isOutput=False)
    wv = nc.declare_dram_parameter("wv", [128, NCH * E1], BF16, isOutput=False)
    bq = nc.declare_dram_parameter("bqb", [128, 512], BF16, isOutput=False)
    bvb = nc.declare_dram_parameter("bvb", [128, E1], F32, isOutput=False)
    out = nc.declare_dram_parameter("out", [E1, H], F32, isOutput=True)

    with tile.TileContext(nc) as tc:
        _body(nc, tc, qk_parms, wq, wk, wv, bq, bvb, out)
    if split_waits:
        _split_multi_waits(nc)
    return nc


_NO_SPLIT_OPCODES = {"Drain", "EventSemaphore", "NoOp", "Call", "ISA",
                     "UnconditionalBranch"}


def _split_multi_waits(nc):
    """walrus (this toolchain) encodes at most ONE sem wait per TPB
    instruction (single NEURON_ISA_TPB_EVENTS slot) and refuses to compile
    instructions carrying more. Tile emits multi-wait sync_info freely, so
    split: keep the first wait on the instruction, hoist the rest onto
    standalone EventSemaphore waits just before it on the same engine."""
    n = 0
    for blk in nc.m.functions[0].blocks:
        new_insts = []
        for inst in blk.instructions:
            si = inst.sync_info
            if (si is not None and si.on_wait and len(si.on_wait) > 1
                    and inst.concise_opcode not in _NO_SPLIT_OPCODES):
                waits = list(si.on_wait)
                for w in waits[:-1]:
                    n += 1
                    es = mybir.InstEventSemaphore(
                        name=f"WSPLIT-{n}", ins=[], outs=[])
                    es.engine = inst.engine
                    es.sync_info = mybir.SyncInfo(on_wait=[w], on_update=[])
                    new_insts.append(es)
                inst.sync_info = mybir.SyncInfo(
                    on_wait=[waits[-1]], on_update=list(si.on_update))
            new_insts.append(inst)
        blk.instructions = new_insts


def _body(nc, tc, parms, wq, wk, wv, bq, bvb, out):
    with (
        tc.tile_pool(name="consts", bufs=1) as cst,
        tc.tile_pool(name="raw", bufs=1) as raw,
        tc.tile_pool(name="proj", bufs=1) as proj,
        tc.tile_pool(name="pt", bufs=8) as ptp,
        tc.tile_pool(name="ob", bufs=2) as obp,
        tc.tile_pool(name="ps", bufs=2, space="PSUM") as ps,
        tc.tile_pool(name="psc", bufs=2, space="PSUM") as psc,
        tc.tile_pool(name="pso", bufs=2, space="PSUM") as pso,
    ):
        # consts ride the gpsimd SWDGE ring so they never add fixed
        # per-DMA latency ahead of the big input loads on the HWDGE rings
        wk_sb = cst.tile([128, NCH * E2], BF16, tag="wk")
        nc.gpsimd.dma_start(out=wk_sb, in_=wk[:, :])
        wq_sb = cst.tile([128, NCH * E2], BF16, tag="wq")
        nc.gpsimd.dma_start(out=wq_sb, in_=wq[:, :])
        # preload the exp table set off the critical path (first real exp
        # otherwise eats the ~2.7us ACT_TABLE_LOAD mid-pipeline); hangs off
        # the first const so the table DMA fires immediately
        scr = cst.tile([1, 8], F32, tag="scr")
        nc.scalar.activation(scr[:, :], wk_sb[0:1, 0:8],
                             mybir.ActivationFunctionType.Exp)
        bq_sb = cst.tile([128, 512], BF16, tag="bq")
        nc.gpsimd.dma_start(out=bq_sb, in_=bq[:, :])
        wv_sb = cst.tile([128, NCH * E1], BF16, tag="wv")
        nc.gpsimd.dma_start(out=wv_sb, in_=wv[:, :])
        bvb_sb = cst.tile([128, E1], F32, tag="bvb")
        nc.gpsimd.dma_start(out=bvb_sb, in_=bvb[:, :])

        # raw inputs, host-swizzled to the SBUF layout: each DMA is a
        # flat [128, chunk*cols] contiguous-per-partition transfer.
        # sync carries qT+kT (critical chain to the first scores matmul);
        # scalar carries vT, gated behind ka so v doesn't steal SDMA
        # bandwidth from the first projections.
        tiles = {}
        insts = {}

        def load(eng, name, w, dt):
            t = raw.tile([128, NCH, w], dt, tag=name)
            insts[name] = eng.dma_start(out=t, in_=parms[name][:, :, :])
            tiles[name] = t

        load(nc.sync, "qa", 512, FP8)
        load(nc.sync, "ka", 512, FP8)
        load(nc.sync, "kb", 512, FP8)
        load(nc.sync, "kc", 1024, FP8)
        load(nc.sync, "qb", 1536, FP8)
        load(nc.sync, "qc", 2048, FP8)
        load(nc.scalar, "va", 512, BF16)
        load(nc.scalar, "vb", 1536, BF16)

        def qt_slice(c, blk):
            if blk == 0:
                return tiles["qa"][:, c, :]
            if blk < 4:
                return tiles["qb"][:, c, (blk - 1) * 512:blk * 512]
            return tiles["qc"][:, c, (blk - 4) * 512:(blk - 3) * 512]

        def kt_slice(c, blk):       # 512-col K projection block
            if blk == 0:
                return tiles["ka"][:, c, :]
            if blk == 1:
                return tiles["kb"][:, c, :]
            return tiles["kc"][:, c, (blk - 2) * 512:(blk - 1) * 512]

        def vt_slice(c, t):         # 128-col V tile
            if t < 4:
                return tiles["va"][:, c, t * 128:(t + 1) * 128]
            return tiles["vb"][:, c, (t - 4) * 128:(t - 3) * 128]

        # projected tensors; Q.T/K.T have rows 0..63 duplicated into
        # 64..127 (written in one pass via the doubled stationaries) so
        # the scores matmuls can row-pack both PE array halves
        QT2 = proj.tile([128, H], BF16, tag="QT2")
        KT2 = proj.tile([128, KS], BF16, tag="KT2")
        Vp = proj.tile([128, NKV, E1], BF16, tag="Vp")

        def q_proj(blk):
            acc = ps.tile([128, 512], F32, tag="ps_main")
            sl = slice(blk * 512, (blk + 1) * 512)
            for c in range(NCH):
                nc.tensor.matmul(
                    acc[:, :], wq_sb[:, c * E2:(c + 1) * E2], qt_slice(c, blk),
                    start=(c == 0), stop=(c == NCH - 1),
                )
            nc.vector.tensor_add(QT2[:, sl], acc[:, :], bq_sb[:, :])

        def k_proj(blk):
            acc = ps.tile([128, 512], F32, tag="ps_main")
            sl = slice(blk * 512, (blk + 1) * 512)
            for c in range(NCH):
                nc.tensor.matmul(
                    acc[:, :], wk_sb[:, c * E2:(c + 1) * E2], kt_slice(c, blk),
                    start=(c == 0), stop=(c == NCH - 1),
                )
            nc.vector.tensor_copy(KT2[:, sl], acc[:, :])

        def v_proj(t):
            acc = ps.tile([128, E1], F32, tag="ps_main")
            for c in range(NCH):
                nc.tensor.matmul(
                    acc[:, :], vt_slice(c, t),
                    wv_sb[:, c * E1:(c + 1) * E1],
                    start=(c == 0), stop=(c == NCH - 1),
                )
            nc.vector.tensor_add(Vp[:, t, :], acc[:, :], bvb_sb[:, :])

        # ---- software-pipelined attention: 64 pair steps, skew 1 ----
        sc_t = [None] * NSTEP
        pt_t = [None] * NSTEP
        acc = {}

        def scores(i):
            blk, p = divmod(i, NPAIR)
            sq = slice(blk * QBLK, (blk + 1) * QBLK)
            sc = psc.tile([128, 2 * QBLK], F32, tag="ps_sc")
            nc.tensor.matmul(
                sc[:, 0:QBLK],
                KT2[0:E, (2 * p) * 128:(2 * p + 1) * 128],
                QT2[0:E, sq],
                start=True, stop=True, tile_position=(0, 0),
            )
            nc.tensor.matmul(
                sc[:, QBLK:2 * QBLK],
                KT2[E:2 * E, (2 * p + 1) * 128:(2 * p + 2) * 128],
                QT2[E:2 * E, sq],
                start=True, stop=True, tile_position=(64, 0),
            )
            sc_t[i] = sc

        def expq(i):
            pt = ptp.tile([128, 2 * QBLK], BF16, tag="pt")
            nc.scalar.activation(
                pt[:, :], sc_t[i][:, :], mybir.ActivationFunctionType.Exp,
                scale=0.125,
            )
            pt_t[i] = pt

        def av(i):
            blk, p = divmod(i, NPAIR)
            if p == 0:
                acc[blk] = pso.tile([E1, QBLK], F32, tag="ps_out",
                                    name=f"acc{blk}")
            a = acc[blk]
            nc.tensor.matmul(
                a[:, :], Vp[:, 2 * p, :], pt_t[i][:, 0:QBLK],
                start=(p == 0), stop=False,
            )
            nc.tensor.matmul(
                a[:, :], Vp[:, 2 * p + 1, :], pt_t[i][:, QBLK:2 * QBLK],
                start=False, stop=(p == NPAIR - 1),
            )
            if p == NPAIR - 1:
                sq = slice(blk * QBLK, (blk + 1) * QBLK)
                ob = obp.tile([E1, QBLK], F32, tag="ob")
                nc.vector.tensor_copy(ob[:, :], a[:, :])
                nc.scalar.dma_start(out=out[:, sq], in_=ob[:, :])

        # filler (projection) work attached ahead of specific steps so it
        # lands in the PE's exp-wait slack and tracks DMA arrival order
        fillers = {
            0: [lambda: v_proj(0), lambda: v_proj(1), lambda: v_proj(2),
                lambda: v_proj(3)],
            1: [lambda: k_proj(1), lambda: v_proj(4), lambda: v_proj(5)],
            2: [lambda: v_proj(6), lambda: v_proj(7)],
            3: [lambda: k_proj(2), lambda: v_proj(8), lambda: v_proj(9)],
            4: [lambda: q_proj(1), lambda: v_proj(10), lambda: v_proj(11)],
            5: [lambda: k_proj(3), lambda: v_proj(12), lambda: v_proj(13)],
            6: [lambda: v_proj(14), lambda: v_proj(15)],
        }
        for b in range(2, NBLK):
            fillers.setdefault(8 * (b - 1) + 4, []).append(
                lambda b=b: q_proj(b))

        # HAM warmup: keep the PE busy on throwaway matmuls while the
        # first input DMAs land, so the projections run at 2.4 GHz
        warm = ps.tile([128, 512], F32, tag="ps_main")
        for w in range(6):
            nc.tensor.matmul(warm[:, :], wk_sb[:, 0:128], wk_sb[:, 0:512],
                             start=(w == 0), stop=(w == 5))

        q_proj(0)
        k_proj(0)
        scores(0)
        expq(0)
        for f in fillers.get(0, []):
            f()
        for i in range(1, NSTEP):
            scores(i)
            expq(i)
            for f in fillers.get(i, []):
                f()
            av(i - 1)
        av(NSTEP - 1)


_CACHED_NC = None


def _get_nc():
    global _CACHED_NC
    if _CACHED_NC is None:
        _CACHED_NC = _build_bass()
    return _CACHED_NC


def _swizzle_w(w: np.ndarray, double: bool = False) -> np.ndarray:
    """[512, width] -> [128, NCH*width] with chunk-major free dim.
    double=True emits [W|W] chunks ([128, NCH*2*width]) so one matmul
    writes the projection into both partition halves."""
    width = w.shape[1]
    c = w.reshape(NCH, 128, width)
    if double:
        c = np.concatenate([c, c], axis=2)
        width *= 2
    return np.ascontiguousarray(
        c.transpose(1, 0, 2).reshape(128, NCH * width)
    ).astype(ml_dtypes.bfloat16)


def _chunk3d(xT: np.ndarray, splits, dt) -> dict:
    """[512, N] (transposed input) -> per-split [128, NCH, w] arrays in
    the exact SBUF tile layout (partition p, chunk c) = row c*128+p."""
    x = np.asarray(xT, np.float32).reshape(NCH, 128, xT.shape[1])
    outmaps = {}
    c0 = 0
    for name, w in splits:
        outmaps[name] = np.ascontiguousarray(
            x[:, :, c0:c0 + w].transpose(1, 0, 2)).astype(dt)
        c0 += w
    return outmaps


def _make_in_maps(q, k, v, Wq, bq, Wk, bk, Wv, bv):
    del bk  # constant along the kv axis -> softmax-invariant, dropped
    bf = ml_dtypes.bfloat16
    f8 = ml_dtypes.float8_e3m4
    wq_s = _swizzle_w(np.asarray(Wq, np.float32), double=True)
    wk_s = _swizzle_w(np.asarray(Wk, np.float32), double=True)
    wv_aug = np.concatenate(
        [np.asarray(Wv, np.float32), np.zeros((D, 1), np.float32)], axis=1
    )
    wv_s = _swizzle_w(wv_aug)
    bq_col = np.asarray(bq, np.float32).reshape(E, 1)
    bq_a = np.ascontiguousarray(np.broadcast_to(
        np.concatenate([bq_col, bq_col], axis=0), (2 * E, 512))).astype(bf)
    bvb_row = np.concatenate([np.asarray(bv, np.float32), [1.0]]).astype(np.float32)
    bvb_a = np.ascontiguousarray(np.broadcast_to(bvb_row, (128, E1)))

    in_maps = []
    for core in range(N_CORES):
        b, h = core // 2, core % 2
        m = {"wq": wq_s, "wk": wk_s, "wv": wv_s, "bqb": bq_a, "bvb": bvb_a}
        m.update(_chunk3d(np.asarray(q[b], np.float32).T, Q_SPLITS, f8))
        m.update(_chunk3d(
            np.asarray(k[b, h * KS:(h + 1) * KS, :], np.float32).T,
            K_SPLITS, f8))
        m.update(_chunk3d(
            np.asarray(v[b, h * KS:(h + 1) * KS, :], np.float32).T,
            V_SPLITS, bf))
        in_maps.append(m)
    return in_maps


def _unshard(results) -> np.ndarray:
    final = np.empty((B, S, E), np.float32)
    for b in range(B):
        o = (np.asarray(results[2 * b]["out"], np.float32)
             + np.asarray(results[2 * b + 1]["out"], np.float32))  # [65, S]
        final[b] = (o[:E] / o[E:E + 1]).T
    return final


def kernel(q, k, v, Wq, bq, Wk, bk, Wv, bv, _trace=False):
    nc = _get_nc()
    in_maps = _make_in_maps(q, k, v, Wq, bq, Wk, bk, Wv, bv)
    res = run_bass_kernel_spmd(nc, in_maps, core_ids=list(range(N_CORES)),
                               trace=_trace)
    outp = _unshard(res.results)
    if _trace:
        kernel.last_result = res
    return outp
